# revision 1
# baseline (speedup 1.0000x reference)
"""AttnBlock (GroupNorm -> single-head attention over 4096 tokens -> proj
-> residual) on 8 Trainium2 NeuronCores.

Sharding: data-parallel over batch (4) x query-token-half (2) = 8 cores.
Each core gets its batch's full x [512, 4096] (for group stats, K, V) and
its query half xh [512, 2048]; it computes K/V for all 4096 tokens and
attention outputs for its 2048 queries. No collectives needed.

Layouts on core (c = channel, t/j = key token, i = query token):
  hn, k:   [c, t]   (channels on partitions)     scoresT = k^T q directly
  vT:      [t, c]   (tokens on partitions)       AV matmul needs j on K-dim
  scoresT: [j, i]   softmax denominator via DVE adds + ones-matmul bcast
Matmuls run in float32r (TF32-like, full PE rate at free-dim 512).
V bias is folded in via out/Z + bv (softmax weights sum to 1), so V is
computed without bias and normalization happens before the projection.
"""
import numpy as np

C = 512
N_TOK = 4096
HALF = 2048
B = 4
N_CORES = 8
NUM_GROUPS = 32
EPS = 1e-6
SCALE = float(C) ** -0.5
GROUP_N = (C // NUM_GROUPS) * N_TOK  # elements per group = 16*4096

_CACHE = {}


def _build_nc():
    from contextlib import ExitStack

    import concourse.bass as bass
    import concourse.mybir as mybir
    import concourse.tile as tile

    f32 = mybir.dt.float32
    f32r = mybir.dt.float32r
    AF = mybir.ActivationFunctionType
    ALU = mybir.AluOpType
    AX = mybir.AxisListType

    nc = bass.Bass()
    x_ext = nc.declare_dram_parameter("x", [C, N_TOK], f32, isOutput=False)
    xh_ext = nc.declare_dram_parameter("xh", [C, HALF], f32, isOutput=False)
    wqT_ext = nc.declare_dram_parameter("wqT", [C, C], f32r, isOutput=False)
    wkT_ext = nc.declare_dram_parameter("wkT", [C, C], f32r, isOutput=False)
    wvT_ext = nc.declare_dram_parameter("wvT", [C, C], f32r, isOutput=False)
    wpT_ext = nc.declare_dram_parameter("wpT", [C, C], f32r, isOutput=False)
    bq_ext = nc.declare_dram_parameter("bq", [C], f32, isOutput=False)
    bk_ext = nc.declare_dram_parameter("bk", [C], f32, isOutput=False)
    bv_ext = nc.declare_dram_parameter("bv", [C], f32, isOutput=False)
    bp_ext = nc.declare_dram_parameter("bp", [C], f32, isOutput=False)
    gam_ext = nc.declare_dram_parameter("gamma", [C], f32, isOutput=False)
    bet_ext = nc.declare_dram_parameter("beta", [C], f32, isOutput=False)
    sel_ext = nc.declare_dram_parameter("sel", [128, 8], f32, isOutput=False)
    selT_ext = nc.declare_dram_parameter("selT", [8, 128], f32, isOutput=False)
    ones_ext = nc.declare_dram_parameter("ones", [128, 128], f32, isOutput=False)
    y_ext = nc.declare_dram_parameter("y", [C, HALF], f32, isOutput=True)
    q_dram = nc.dram_tensor("q_stage", [C, HALF], f32r)

    x_r = x_ext.rearrange("(ct p) n -> p ct n", p=128)
    xh_r = xh_ext.rearrange("(ct p) n -> p ct n", p=128)
    y_r = y_ext.rearrange("(ct p) n -> p ct n", p=128)
    q_r = q_dram.ap().rearrange("(ct p) n -> p ct n", p=128)

    with tile.TileContext(nc) as tc, ExitStack() as top:
        consts = top.enter_context(tc.tile_pool(name="consts", bufs=1))
        big = top.enter_context(tc.tile_pool(name="big", bufs=1))

        k_sb = big.tile([128, 4, N_TOK], f32r, name="k_sb")
        vT_sb = big.tile([128, 32, C], f32r, name="vT_sb")
        wp_sb = big.tile([128, 4, C], f32r, name="wp_sb")
        nc.sync.dma_start(out=wp_sb, in_=wpT_ext.rearrange("(ci p) o -> p ci o", p=128))

        sel_sb = consts.tile([128, 8], f32, name="sel_sb")
        nc.sync.dma_start(out=sel_sb, in_=sel_ext[:])
        selT_sb = consts.tile([8, 128], f32, name="selT_sb")
        nc.sync.dma_start(out=selT_sb, in_=selT_ext[:])
        ones_sb = consts.tile([128, 128], f32, name="ones_sb")
        nc.sync.dma_start(out=ones_sb, in_=ones_ext[:])

        def load_bias(name, ext):
            t = consts.tile([128, 4], f32, name=name)
            nc.sync.dma_start(out=t, in_=ext.rearrange("(ct p) -> p ct", p=128))
            return t

        bq_sb = load_bias("bq_sb", bq_ext)
        bk_sb = load_bias("bk_sb", bk_ext)
        bv_sb = load_bias("bv_sb", bv_ext)
        bp_sb = load_bias("bp_sb", bp_ext)
        gam_sb = load_bias("gam_sb", gam_ext)
        bet_sb = load_bias("bet_sb", bet_ext)

        # ---------------- Stage A: group-norm statistics ----------------
        sc = consts.tile([128, 4], f32, name="sc")
        bi = consts.tile([128, 4], f32, name="bi")
        with ExitStack() as stA:
            pa = stA.enter_context(tc.tile_pool(name="pa", bufs=2))
            psA = stA.enter_context(tc.tile_pool(name="psA", bufs=2, space="PSUM"))
            sums = consts.tile([128, 4, 2], f32, name="sums")
            for ct in range(4):
                xs = pa.tile([128, N_TOK], f32, name="xs")
                nc.sync.dma_start(out=xs, in_=x_ext[ct * 128:(ct + 1) * 128, :])
                sq = pa.tile([128, N_TOK], f32, name="sq")
                nc.scalar.activation(out=sq, in_=xs, func=AF.Square)
                nc.vector.tensor_reduce(out=sums[:, ct, 0:1], in_=xs, axis=AX.X, op=ALU.add)
                nc.vector.tensor_reduce(out=sums[:, ct, 1:2], in_=sq, axis=AX.X, op=ALU.add)
            gp = psA.tile([8, 8], f32, name="gp")
            for ct in range(4):
                nc.tensor.matmul(gp[:, ct * 2:(ct + 1) * 2], sel_sb, sums[:, ct, :],
                                 start=True, stop=True)
            gst = consts.tile([8, 4, 2], f32, name="gst")
            nc.vector.tensor_copy(gst, gp)
            inv_n = 1.0 / GROUP_N
            m_t = consts.tile([8, 4], f32, name="m_t")
            e2_t = consts.tile([8, 4], f32, name="e2_t")
            nc.vector.tensor_scalar_mul(out=m_t, in0=gst[:, :, 0], scalar1=inv_n)
            nc.vector.tensor_scalar_mul(out=e2_t, in0=gst[:, :, 1], scalar1=inv_n)
            msq = consts.tile([8, 4], f32, name="msq")
            nc.vector.tensor_mul(msq, m_t, m_t)
            ve = consts.tile([8, 4], f32, name="ve")
            nc.vector.tensor_tensor(out=ve, in0=e2_t, in1=msq, op=ALU.subtract)
            nc.vector.tensor_scalar_add(ve, ve, EPS)
            sd = consts.tile([8, 4], f32, name="sd")
            nc.scalar.activation(out=sd, in_=ve, func=AF.Sqrt)
            r0 = consts.tile([8, 4], f32, name="r0")
            nc.vector.reciprocal(r0, sd)
            # one Newton step: r = r0*(1.5 - 0.5*ve*r0^2)
            t1 = consts.tile([8, 4], f32, name="t1")
            nc.vector.tensor_mul(t1, r0, r0)
            nc.vector.tensor_mul(t1, t1, ve)
            nc.vector.tensor_scalar(out=t1, in0=t1, scalar1=-0.5, scalar2=1.5,
                                    op0=ALU.mult, op1=ALU.add)
            gmr = consts.tile([8, 4, 2], f32, name="gmr")
            nc.vector.tensor_mul(gmr[:, :, 1], r0, t1)
            nc.vector.tensor_copy(gmr[:, :, 0], m_t)
            chan = consts.tile([128, 4, 2], f32, name="chan")
            for ct in range(4):
                chp = psA.tile([128, 2], f32, name="chp")
                nc.tensor.matmul(chp, selT_sb, gmr[:, ct, :], start=True, stop=True)
                nc.vector.tensor_copy(chan[:, ct, :], chp)
            nc.vector.tensor_tensor(out=sc, in0=chan[:, :, 1], in1=gam_sb, op=ALU.mult)
            tb_ = consts.tile([128, 4], f32, name="tb_")
            nc.vector.tensor_tensor(out=tb_, in0=chan[:, :, 0], in1=sc, op=ALU.mult)
            nc.vector.tensor_tensor(out=bi, in0=bet_sb, in1=tb_, op=ALU.subtract)

        # ---------------- Stage B: hn -> k, vT, q ----------------
        with ExitStack() as stB:
            pb = stB.enter_context(tc.tile_pool(name="pb", bufs=2))
            psB = stB.enter_context(tc.tile_pool(name="psB", bufs=3, space="PSUM"))
            with ExitStack() as stB1:
                wkv = stB1.enter_context(tc.tile_pool(name="wkv", bufs=1))
                wk_sb = wkv.tile([128, 4, C], f32r, name="wk_sb")
                nc.sync.dma_start(out=wk_sb, in_=wkT_ext.rearrange("(ci p) o -> p ci o", p=128))
                wv_sb = wkv.tile([128, 4, C], f32r, name="wv_sb")
                nc.sync.dma_start(out=wv_sb, in_=wvT_ext.rearrange("(ci p) o -> p ci o", p=128))
                for tb in range(8):
                    xb_t = pb.tile([128, 4, 512], f32, name="xb_t")
                    nc.sync.dma_start(out=xb_t, in_=x_r[:, :, tb * 512:(tb + 1) * 512])
                    hn_t = pb.tile([128, 4, 512], f32r, name="hn_t")
                    for ct in range(4):
                        nc.scalar.activation(out=hn_t[:, ct, :], in_=xb_t[:, ct, :],
                                             func=AF.Identity,
                                             scale=sc[:, ct:ct + 1], bias=bi[:, ct:ct + 1])
                    for co in range(4):
                        kp = psB.tile([128, 512], f32, name="kp")
                        for ci in range(4):
                            nc.tensor.matmul(kp, wk_sb[:, ci, co * 128:(co + 1) * 128],
                                             hn_t[:, ci, :], start=(ci == 0), stop=(ci == 3))
                        nc.scalar.activation(out=k_sb[:, co, tb * 512:(tb + 1) * 512],
                                             in_=kp, func=AF.Identity,
                                             bias=bk_sb[:, co:co + 1])
                    for tt in range(4):
                        vp = psB.tile([128, 512], f32, name="vp")
                        for ci in range(4):
                            nc.tensor.matmul(vp, hn_t[:, ci, tt * 128:(tt + 1) * 128],
                                             wv_sb[:, ci, :], start=(ci == 0), stop=(ci == 3))
                        nc.vector.tensor_copy(vT_sb[:, tb * 4 + tt, :], vp)
            with ExitStack() as stB2:
                wqp = stB2.enter_context(tc.tile_pool(name="wqp", bufs=1))
                wq_sb = wqp.tile([128, 4, C], f32r, name="wq_sb")
                nc.sync.dma_start(out=wq_sb, in_=wqT_ext.rearrange("(ci p) o -> p ci o", p=128))
                for qb in range(4):
                    xq_t = pb.tile([128, 4, 512], f32, name="xq_t", tag="xb_t")
                    nc.sync.dma_start(out=xq_t, in_=xh_r[:, :, qb * 512:(qb + 1) * 512])
                    hq_t = pb.tile([128, 4, 512], f32r, name="hq_t", tag="hn_t")
                    for ct in range(4):
                        nc.scalar.activation(out=hq_t[:, ct, :], in_=xq_t[:, ct, :],
                                             func=AF.Identity,
                                             scale=sc[:, ct:ct + 1], bias=bi[:, ct:ct + 1])
                    qs_t = pb.tile([128, 4, 512], f32r, name="qs_t")
                    for co in range(4):
                        qp = psB.tile([128, 512], f32, name="qp", tag="kp")
                        for ci in range(4):
                            nc.tensor.matmul(qp, wq_sb[:, ci, co * 128:(co + 1) * 128],
                                             hq_t[:, ci, :], start=(ci == 0), stop=(ci == 3))
                        nc.scalar.activation(out=qs_t[:, co, :], in_=qp, func=AF.Identity,
                                             bias=bq_sb[:, co:co + 1])
                    nc.sync.dma_start(out=q_r[:, :, qb * 512:(qb + 1) * 512], in_=qs_t)

        # ---------------- Stage C: attention + proj + residual ----------------
        with ExitStack() as stC:
            pc = stC.enter_context(tc.tile_pool(name="pc", bufs=1))
            pc2 = stC.enter_context(tc.tile_pool(name="pc2", bufs=2))
            ps_acc = stC.enter_context(tc.tile_pool(name="ps_acc", bufs=1, space="PSUM"))
            ps_sT = stC.enter_context(tc.tile_pool(name="ps_sT", bufs=2, space="PSUM"))
            for ib in range(4):
                qb_sb = pc.tile([128, 4, 512], f32r, name="qb_sb")
                nc.sync.dma_start(out=qb_sb, in_=q_r[:, :, ib * 512:(ib + 1) * 512])
                Zp = pc.tile([128, 512], f32, name="Zp")
                nc.vector.memset(Zp, 0.0)
                oap = ps_acc.tile([128, 4, 512], f32, name="oap", tag="acc")
                for jg in range(16):
                    sTp = ps_sT.tile([128, 2, 512], f32, name="sTp", tag="sT")
                    for jt2 in range(2):
                        jt = jg * 2 + jt2
                        for ci in range(4):
                            nc.tensor.matmul(sTp[:, jt2, :],
                                             k_sb[:, ci, jt * 128:(jt + 1) * 128],
                                             qb_sb[:, ci, :],
                                             start=(ci == 0), stop=(ci == 3))
                    ptg = pc2.tile([128, 2, 512], f32r, name="ptg")
                    nc.scalar.activation(out=ptg, in_=sTp, func=AF.Exp, scale=SCALE)
                    zt = pc2.tile([128, 512], f32, name="zt")
                    nc.vector.tensor_add(zt, ptg[:, 0, :].bitcast(f32),
                                         ptg[:, 1, :].bitcast(f32))
                    nc.vector.tensor_add(Zp, Zp, zt)
                    for ct in range(4):
                        for jt2 in range(2):
                            jt = jg * 2 + jt2
                            nc.tensor.matmul(oap[:, ct, :],
                                             vT_sb[:, jt, ct * 128:(ct + 1) * 128],
                                             ptg[:, jt2, :],
                                             start=(jg == 0 and jt2 == 0),
                                             stop=(jg == 15 and jt2 == 1))
                zbp = ps_sT.tile([128, 512], f32, name="zbp", tag="sT")
                nc.tensor.matmul(zbp, ones_sb, Zp, start=True, stop=True)
                rz = pc.tile([128, 512], f32, name="rz")
                nc.vector.reciprocal(rz, zbp)
                oa_sb = pc.tile([128, 4, 512], f32r, name="oa_sb")
                for ct in range(4):
                    t1c = pc2.tile([128, 512], f32, name="t1c")
                    nc.vector.tensor_mul(t1c, oap[:, ct, :], rz)
                    nc.vector.tensor_scalar(out=oa_sb[:, ct, :], in0=t1c,
                                            scalar1=bv_sb[:, ct:ct + 1], scalar2=None,
                                            op0=ALU.add)
                up = ps_acc.tile([128, 4, 512], f32, name="up", tag="acc")
                for co in range(4):
                    for ci in range(4):
                        nc.tensor.matmul(up[:, co, :],
                                         wp_sb[:, ci, co * 128:(co + 1) * 128],
                                         oa_sb[:, ci, :],
                                         start=(ci == 0), stop=(ci == 3))
                xh_b = pc.tile([128, 4, 512], f32, name="xh_b")
                nc.sync.dma_start(out=xh_b, in_=xh_r[:, :, ib * 512:(ib + 1) * 512])
                y_sb = pc.tile([128, 4, 512], f32, name="y_sb")
                for ct in range(4):
                    t2c = pc2.tile([128, 512], f32, name="t2c")
                    nc.vector.tensor_scalar(out=t2c, in0=up[:, ct, :],
                                            scalar1=bp_sb[:, ct:ct + 1], scalar2=None,
                                            op0=ALU.add)
                    nc.vector.tensor_add(y_sb[:, ct, :], t2c, xh_b[:, ct, :])
                nc.sync.dma_start(out=y_r[:, :, ib * 512:(ib + 1) * 512], in_=y_sb)

    _split_excess_waits(nc)
    return nc


def _split_excess_waits(nc, limit=1):
    """walrus in this container accepts at most one sync-wait per
    instruction; hoist excess waits onto preceding same-engine NoOps."""
    import bass_rust
    import concourse.mybir as mybir

    n = 0
    for f in nc.m.functions:
        for bb in f.blocks:
            out = []
            for inst in bb.instructions:
                si = inst.sync_info
                waits = list(si.on_wait) if si and si.on_wait else []
                if len(waits) > limit:
                    excess, keep = waits[:-limit], waits[-limit:]
                    for ci in range(0, len(excess), limit):
                        nop = mybir.InstNoOp(name=f"{inst.name}-ws{ci}", ins=[], outs=[])
                        nop.engine = inst.engine
                        nop.sync_info = bass_rust.SyncInfo(
                            on_wait=list(excess[ci:ci + limit]), on_update=[])
                        out.append(nop)
                        n += 1
                    inst.sync_info = bass_rust.SyncInfo(
                        on_wait=list(keep),
                        on_update=list(si.on_update) if si.on_update else [])
                out.append(inst)
            bb.instructions[:] = out
    return n


def _get_nc():
    if "nc" not in _CACHE:
        _CACHE["nc"] = _build_nc()
    return _CACHE["nc"]


def _host_constants():
    sel = np.zeros((128, 8), np.float32)
    for p in range(128):
        sel[p, p // 16] = 1.0
    selT = np.ascontiguousarray(sel.T)
    ones = np.ones((128, 128), np.float32)
    return sel, selT, ones


def kernel(x, norm_gamma, norm_beta, wq, bq, wk, bk, wv, bv, wp, bp):
    from concourse.bass_utils import run_bass_kernel_spmd

    nc = _get_nc()
    x = np.asarray(x, dtype=np.float32)
    sel, selT, ones = _host_constants()
    common = {
        "wqT": np.ascontiguousarray(np.asarray(wq, np.float32).T),
        "wkT": np.ascontiguousarray(np.asarray(wk, np.float32).T),
        "wvT": np.ascontiguousarray(np.asarray(wv, np.float32).T),
        "wpT": np.ascontiguousarray(np.asarray(wp, np.float32).T),
        "bq": np.asarray(bq, np.float32), "bk": np.asarray(bk, np.float32),
        "bv": np.asarray(bv, np.float32), "bp": np.asarray(bp, np.float32),
        "gamma": np.asarray(norm_gamma, np.float32),
        "beta": np.asarray(norm_beta, np.float32),
        "sel": sel, "selT": selT, "ones": ones,
    }
    in_maps = []
    for core in range(N_CORES):
        b, qh = core // 2, core % 2
        xb = np.ascontiguousarray(x[b].reshape(C, N_TOK))
        xh = np.ascontiguousarray(xb[:, qh * HALF:(qh + 1) * HALF])
        in_maps.append({"x": xb, "xh": xh, **common})
    res = run_bass_kernel_spmd(nc, in_maps, list(range(N_CORES)))
    out = np.empty((B, C, N_TOK), np.float32)
    for core in range(N_CORES):
        b, qh = core // 2, core % 2
        out[b, :, qh * HALF:(qh + 1) * HALF] = res.results[core]["y"]
    return out.reshape(B, C, 64, 64)
